# revision 3
# baseline (speedup 1.0000x reference)
"""Trainium2 Bass kernel for nn_NewGPTEMA: per-channel damped-EMA causal conv.

Math: y[b,l,d] = sum_m w[d,m] * x[b,l-m,d], where
w[d,m] = (1/sqrt(D)) * sum_n gamma[d,n] * sigmoid(delta[d,n])^m.
sigmoid(delta) with delta ~ N(0,0.2^2) is bounded well away from 1, so the
EMA kernel decays below fp32 resolution within K=64 taps -> exact-to-fp32
banded FIR instead of the reference's length-8192 FFT conv.

Implementation: D-sharded across 8 cores (256 ch/core). Host precomputes,
per channel, two 128x128 Toeplitz matrices (intra-block band + previous-block
halo band). On-device, each output block of 128 positions is two accumulating
TensorE matmuls: psum[l, t] = sum_j A[j,l] x[t,j] + sum_j H[j,l] x[t-1,j],
with data kept in natural [pos, ch] layout throughout (no transposes).
"""

import math
from contextlib import ExitStack

import numpy as np

import concourse.bacc as bacc
import concourse.tile as tile
from concourse import mybir
from concourse.bass_utils import run_bass_kernel_spmd

B, L, D = 4, 4096, 2048
NCORES = 8
DC = D // NCORES          # 256 channels per core
K = 64                    # truncated EMA tap count
PC = 128                  # positions per block
NBLK = L // PC            # 32 blocks per batch
CH_PHASE = 128            # channels processed per phase
F32 = mybir.dt.float32

_CACHE: dict = {}


def _build_taps(delta: np.ndarray, gamma: np.ndarray) -> np.ndarray:
    """(D, K) float32 FIR taps from the EMA params, computed in float64."""
    p = 1.0 / (1.0 + np.exp(-delta[:, :, 0].astype(np.float64)))   # (D, N)
    g = gamma[:, :, 0].astype(np.float64) / math.sqrt(D)           # (D, N)
    powers = p[:, :, None] ** np.arange(K, dtype=np.float64)       # (D, N, K)
    return (g[:, :, None] * powers).sum(axis=1).astype(np.float32)  # (D, K)


def _build_toeplitz(taps: np.ndarray) -> np.ndarray:
    """(D, 2, PC, PC) float32: [c,0]=intra band A[j,l]=w[l-j],
    [c,1]=halo band H[j,l]=w[PC+l-j]."""
    jj, ll = np.meshgrid(np.arange(PC), np.arange(PC), indexing="ij")
    d1 = ll - jj
    m1 = (d1 >= 0) & (d1 < K)
    d2 = PC + ll - jj
    m2 = (d2 >= 0) & (d2 < K)
    A = np.where(m1, taps[:, np.clip(d1, 0, K - 1)], np.float32(0.0))
    H = np.where(m2, taps[:, np.clip(d2, 0, K - 1)], np.float32(0.0))
    return np.stack([A, H], axis=1).astype(np.float32)


def _build_program():
    if "nc" in _CACHE:
        return _CACHE["nc"]
    nc = bacc.Bacc(
        "TRN2",
        target_bir_lowering=False,
        debug=False,
        enable_asserts=False,
        num_devices=NCORES,
    )
    x_ap = nc.dram_tensor("x", [B, L, DC], F32, kind="ExternalInput").ap()
    w_ap = nc.dram_tensor("w", [DC, 2, PC, PC], F32, kind="ExternalInput").ap()
    y_ap = nc.dram_tensor("y", [B, L, DC], F32, kind="ExternalOutput").ap()

    with tile.TileContext(nc) as tc, ExitStack() as ctx:
        xpool = ctx.enter_context(tc.tile_pool(name="xp", bufs=4))
        ypool = ctx.enter_context(tc.tile_pool(name="yp", bufs=4))
        wpool = ctx.enter_context(tc.tile_pool(name="wp", bufs=2))
        pspool = ctx.enter_context(tc.tile_pool(name="ps", bufs=4, space="PSUM"))

        for phase in range(DC // CH_PHASE):
            c0 = phase * CH_PHASE
            xts = []
            yts = []
            for b in range(B):
                xt = xpool.tile([PC, NBLK + 1, CH_PHASE], F32, tag="xt")
                # block 0 = zeros: the halo matmul of each batch's first
                # block then contributes nothing (sequence start).
                nc.vector.memset(xt[:, 0, :], 0.0)
                src = x_ap[b, :, c0:c0 + CH_PHASE].rearrange(
                    "(t p) c -> p t c", p=PC)
                nc.sync.dma_start(xt[:, 1:, :], src)
                xts.append(xt)
                yt = ypool.tile([PC, NBLK, CH_PHASE], F32, tag="yt",
                                name=f"yt_{phase}_{b}")
                yts.append(yt)

            for cg in range(CH_PHASE // 16):
                wt = wpool.tile([PC, 16, 2, PC], F32, tag="wt")
                wsrc = w_ap[c0 + cg * 16:c0 + (cg + 1) * 16].rearrange(
                    "s i j l -> j s i l")
                nc.sync.dma_start(wt[:], wsrc)
                for b in range(B):
                    ps = pspool.tile([PC, 16, NBLK], F32, tag="ps")
                    for cs in range(16):
                        c = cg * 16 + cs
                        nc.tensor.matmul(
                            ps[:, cs, :], lhsT=wt[:, cs, 0, :],
                            rhs=xts[b][:, 1:, c], start=True, stop=False)
                        nc.tensor.matmul(
                            ps[:, cs, :], lhsT=wt[:, cs, 1, :],
                            rhs=xts[b][:, 0:NBLK, c], start=False, stop=True)
                    dst = yts[b][:, :, cg * 16:(cg + 1) * 16].rearrange(
                        "p t c -> p c t")
                    nc.scalar.copy(dst, ps[:])

            for b in range(B):
                dstd = y_ap[b, :, c0:c0 + CH_PHASE].rearrange(
                    "(t p) c -> p t c", p=PC)
                nc.sync.dma_start(dstd, yts[b][:])

    nc.compile()
    _CACHE["nc"] = nc
    return nc


def kernel(hidden_states: np.ndarray, delta: np.ndarray,
           gamma: np.ndarray) -> np.ndarray:
    taps = _build_taps(delta, gamma)
    W = _build_toeplitz(taps)                      # (D, 2, PC, PC)
    nc = _build_program()

    in_maps = []
    for k in range(NCORES):
        sl = slice(k * DC, (k + 1) * DC)
        in_maps.append({
            "x": np.ascontiguousarray(hidden_states[:, :, sl], dtype=np.float32),
            "w": np.ascontiguousarray(W[sl]),
        })
    kres = run_bass_kernel_spmd(nc, in_maps, list(range(NCORES)))
    _CACHE["last_results"] = kres
    res = kres.results
    out = np.concatenate([res[k]["y"] for k in range(NCORES)], axis=2)
    return out.astype(hidden_states.dtype)


# revision 6
# speedup vs baseline: 2.2389x; 2.2389x over previous
"""Trainium2 Bass kernel for nn_NewGPTEMA: per-channel damped-EMA causal conv.

Math: y[b,l,d] = sum_m w[d,m] * x[b,l-m,d], where
w[d,m] = (1/sqrt(D)) * sum_n gamma[d,n] * sigmoid(delta[d,n])^m.
sigmoid(delta) with delta ~ N(0,0.2^2) is bounded well away from 1, so the
EMA kernel decays below fp32 resolution within K=64 taps -> exact-to-fp32
banded FIR instead of the reference's length-8192 FFT conv.

Implementation: D-sharded across 8 cores (256 ch/core). Host precomputes,
per channel, a 128x128 intra-block Toeplitz band A[j,l]=w[l-j] and a packed
64x64 halo band H[j',l]=w[64+l-j'] (the only nonzero corner of the
prev-block matrix). On-device, per channel, two accumulating TensorE
matmuls over all batches at once (free dim = 4b x 32 blocks = 128):
  psum[l, (t,b)]  = sum_j A[j,l] x[b, t*128+j]          (start)
  psum[l<64,...] += sum_j' H[j',l] x[b, (t-1)*128+64+j'] (accumulate)
Data stays in natural [pos, ch] layout (no transposes, no scans).
"""

import math
from contextlib import ExitStack

import numpy as np

import concourse.bacc as bacc
import concourse.tile as tile
from concourse import mybir
from concourse.bass_utils import run_bass_kernel_spmd

B, L, D = 4, 4096, 2048
NCORES = 8
DC = D // NCORES          # 256 channels per core
K = 64                    # truncated EMA tap count
PC = 128                  # positions per block
NBLK = L // PC            # 32 blocks per batch
CH_PHASE = 64             # channels per pipeline phase
CG = 4                    # channels per psum bank / weight tile
F32 = mybir.dt.float32

_CACHE: dict = {}


def _build_taps(delta: np.ndarray, gamma: np.ndarray) -> np.ndarray:
    """(D, K) float32 FIR taps from the EMA params, computed in float64."""
    p = 1.0 / (1.0 + np.exp(-delta[:, :, 0].astype(np.float64)))   # (D, N)
    g = gamma[:, :, 0].astype(np.float64) / math.sqrt(D)           # (D, N)
    powers = p[:, :, None] ** np.arange(K, dtype=np.float64)       # (D, N, K)
    return (g[:, :, None] * powers).sum(axis=1).astype(np.float32)  # (D, K)


def _build_mats(taps: np.ndarray):
    """A: (D, PC, PC) intra band; H: (D, 64, 64) packed halo band."""
    jj, ll = np.meshgrid(np.arange(PC), np.arange(PC), indexing="ij")
    d1 = ll - jj
    A = np.where((d1 >= 0) & (d1 < K), taps[:, np.clip(d1, 0, K - 1)],
                 np.float32(0.0)).astype(np.float32)
    jj2, ll2 = np.meshgrid(np.arange(64), np.arange(64), indexing="ij")
    d2 = 64 + ll2 - jj2
    H = np.where((d2 >= 0) & (d2 < K), taps[:, np.clip(d2, 0, K - 1)],
                 np.float32(0.0)).astype(np.float32)
    return A, H


def _build_program():
    if "nc" in _CACHE:
        return _CACHE["nc"]
    nc = bacc.Bacc(
        "TRN2",
        target_bir_lowering=False,
        debug=False,
        enable_asserts=False,
        num_devices=NCORES,
    )
    x_ap = nc.dram_tensor("x", [B, L, DC], F32, kind="ExternalInput").ap()
    wi_ap = nc.dram_tensor("wi", [DC, PC, PC], F32, kind="ExternalInput").ap()
    wh_ap = nc.dram_tensor("wh", [DC, 64, 64], F32, kind="ExternalInput").ap()
    y_ap = nc.dram_tensor("y", [B, L, DC], F32, kind="ExternalOutput").ap()

    with tile.TileContext(nc) as tc, ExitStack() as ctx:
        xpool = ctx.enter_context(tc.tile_pool(name="xp", bufs=2))
        ypool = ctx.enter_context(tc.tile_pool(name="yp", bufs=2))
        wipool = ctx.enter_context(tc.tile_pool(name="wip", bufs=3))
        whpool = ctx.enter_context(tc.tile_pool(name="whp", bufs=3))
        pspool = ctx.enter_context(tc.tile_pool(name="ps", bufs=4, space="PSUM"))

        for phase in range(DC // CH_PHASE):
            c0 = phase * CH_PHASE
            # x tile: [pos-in-block, t(0=zero pad), b, ch]
            xt = xpool.tile([PC, NBLK + 1, B, CH_PHASE], F32, tag="xt",
                            name=f"xt_{phase}")
            nc.vector.memset(xt[:, 0, :, :], 0.0)
            yt = ypool.tile([PC, NBLK, B, CH_PHASE], F32, tag="yt",
                            name=f"yt_{phase}")
            for b in range(B):
                src = x_ap[b, :, c0:c0 + CH_PHASE].rearrange(
                    "(t p) c -> p t c", p=PC)
                nc.sync.dma_start(xt[:, 1:, b, :], src)

            for cg in range(CH_PHASE // CG):
                wi = wipool.tile([PC, CG, PC], F32, tag="wi",
                                 name=f"wi_{phase}_{cg}")
                # halo weights live on partitions 64..127 so lhsT and rhs
                # share a base partition (engine requirement).
                wh = whpool.tile([PC, CG, 64], F32, tag="wh",
                                 name=f"wh_{phase}_{cg}")
                ca = c0 + cg * CG
                nc.sync.dma_start(wi[:], wi_ap[ca:ca + CG].rearrange(
                    "s j l -> j s l"))
                nc.sync.dma_start(wh[64:128, :, :], wh_ap[ca:ca + CG].rearrange(
                    "s j l -> j s l"))
                ps = pspool.tile([PC, CG, NBLK, B], F32, tag="ps",
                                 name=f"ps_{phase}_{cg}")
                for ci in range(CG):
                    c = cg * CG + ci
                    nc.tensor.matmul(
                        ps[:, ci, :, :], lhsT=wi[:, ci, :],
                        rhs=xt[:, 1:, :, c], start=True, stop=False)
                    nc.tensor.matmul(
                        ps[0:64, ci, :, :], lhsT=wh[64:128, ci, :],
                        rhs=xt[64:128, 0:NBLK, :, c], start=False, stop=True,
                        skip_group_check=True)
                dst = yt[:, :, :, cg * CG:(cg + 1) * CG].rearrange(
                    "p t b c -> p c t b")
                nc.scalar.copy(dst, ps[:])

            for b in range(B):
                dstd = y_ap[b, :, c0:c0 + CH_PHASE].rearrange(
                    "(t p) c -> p t c", p=PC)
                nc.sync.dma_start(dstd, yt[:, :, b, :])

    nc.compile()
    _CACHE["nc"] = nc
    return nc


def kernel(hidden_states: np.ndarray, delta: np.ndarray,
           gamma: np.ndarray) -> np.ndarray:
    taps = _build_taps(delta, gamma)
    A, H = _build_mats(taps)
    nc = _build_program()

    in_maps = []
    for k in range(NCORES):
        sl = slice(k * DC, (k + 1) * DC)
        in_maps.append({
            "x": np.ascontiguousarray(hidden_states[:, :, sl], dtype=np.float32),
            "wi": np.ascontiguousarray(A[sl]),
            "wh": np.ascontiguousarray(H[sl]),
        })
    kres = run_bass_kernel_spmd(nc, in_maps, list(range(NCORES)))
    _CACHE["last_results"] = kres
    res = kres.results
    out = np.concatenate([res[k]["y"] for k in range(NCORES)], axis=2)
    return out.astype(hidden_states.dtype)
